# revision 15
# baseline (speedup 1.0000x reference)
"""Two-layer GraphSAGE (mean aggregation) on 8 Trainium2 NeuronCores.

Strategy (per the sharding hint): nodes + edges sharded by destination
across 8 cores; weights replicated; P2 = h @ W2_l exchanged via chunked
AllGather so layer-2 aggregation can gather any source's row.

Implementation notes (v4, evidence-driven):
  * Aggregation is per-128-edge one-hot matmuls accumulated in PSUM; edges
    are packed into GW-window gather chunks, counts padded to multiples of
    128 per (window, source-block) so every matmul slice targets exactly
    one destination window.
  * gpsimd.dma_gather descriptor generation is the bottleneck; it runs on
    one Q7 core-pair per SWDGE queue, so the 4 per-chunk source-block
    gathers are spread over queues 0-3 (4x parallel generation).
  * Layer-1 rows are gathered in bf16 (256B descriptor floor); layer-2 P2
    rows are bf16 padded to 128 cols (256B).
  * All one-hot matrices of a chunk are built with ONE DVE is_equal over
    broadcast (stride-0) access patterns -- per-slice DVE ops and their
    cross-engine handshakes were the second bottleneck.
  * deg_inv is folded in after aggregation (per window, from a per-core
    [1, npad] table) instead of being baked into the one-hots.
  * R2 (root term of layer 2) round-trips through DRAM to save SBUF.

Self-contained: hardcodes the problem shapes.
"""

import numpy as np

# ---------------------------------------------------------------- config

IN_CH, HIDDEN, OUT_CH = 128, 128, 64
N_NODES, N_EDGES = 100000, 1600000
NCORES = 8
P = 128                      # partitions / window size
L1_RANGE = 25000             # L1 gather block size (int16 limit)
GW = 3                       # windows per gather chunk


def _derive_cfg(n_nodes):
    shard = n_nodes // NCORES
    nwin = (shard + P - 1) // P
    nchunk = 4 if nwin >= 4 else 1          # AllGather chunks
    chunk_wins = (nwin + nchunk - 1) // nchunk
    chunk_rows = []
    for c in range(nchunk):
        lo = c * chunk_wins * P
        hi = min((c + 1) * chunk_wins * P, shard)
        chunk_rows.append(max(hi - lo, 0))
    ngrp1 = (n_nodes + L1_RANGE - 1) // L1_RANGE
    ngc = (nwin + GW - 1) // GW             # gather chunks
    return dict(shard=shard, nwin=nwin, nchunk=nchunk, chunk_wins=chunk_wins,
                chunk_rows=chunk_rows, ngrp1=ngrp1, ngc=ngc)


# ---------------------------------------------------------------- host prep

def _pack(src_loc, grp, win, core, dstl, ngrp, nwin, zero_rows):
    """Pack per-core gather indices + per-slice dst arrays.

    Nesting order: gather-chunk (GW windows) -> grp -> window. Counts per
    (win, grp) are padded to multiples of 128 (max over cores) so slices
    are window-pure; padding edges point at the block's zero row and carry
    dst 999 (matches no one-hot column).
    """
    ngc = (nwin + GW - 1) // GW
    key = (core * nwin + win) * ngrp + grp
    cnt = np.bincount(key, minlength=NCORES * nwin * ngrp
                      ).reshape(NCORES, nwin, ngrp)
    T128 = (cnt.max(axis=0) + P - 1) // P * P          # [nwin, ngrp]
    S = T128 // P
    sumT = int(T128.sum())
    sumS = int(S.sum())

    colT = np.zeros((nwin, ngrp), np.int64)
    colS = np.zeros((nwin, ngrp), np.int64)
    off_t = off_s = 0
    for c in range(ngc):
        for q in range(ngrp):
            for w in range(c * GW, min((c + 1) * GW, nwin)):
                colT[w, q] = off_t
                colS[w, q] = off_s
                off_t += int(T128[w, q])
                off_s += int(S[w, q])
    assert off_t == sumT and off_s == sumS

    idx_all = np.zeros((NCORES, 16, sumT // 16), np.int16)
    dst_all = np.full((NCORES, P, sumS), 999.0, np.float32)

    chunkidx = win // GW
    order = np.lexsort((win, grp, chunkidx, core))
    so, go, wo, co = (a[order] for a in (src_loc, grp, win, core))
    dl_o = dstl[order]
    e0 = 0
    for ci in range(NCORES):
        for c in range(ngc):
            for q in range(ngrp):
                for w in range(c * GW, min((c + 1) * GW, nwin)):
                    k = int(cnt[ci, w, q])
                    t = int(T128[w, q])
                    if t == 0:
                        assert k == 0
                        continue
                    buf = np.full(t, zero_rows[q], np.int32)
                    buf[:k] = so[e0:e0 + k]
                    db = np.full(t, 999.0, np.float32)
                    db[:k] = dl_o[e0:e0 + k]
                    e0 += k
                    base = int(colT[w, q]) // 16
                    idx_all[ci, :, base:base + t // 16] = (
                        buf.reshape(t // 16, 16).T)
                    ns = t // P
                    sbase = int(colS[w, q])
                    dst_all[ci, :, sbase:sbase + ns] = db.reshape(ns, P).T
    assert e0 == len(order)
    idx_rep = np.tile(idx_all, (1, 8, 1))
    return idx_rep, dst_all, T128, S, colT, colS, sumT, sumS


def _preprocess(x, edge_index, cfg):
    import ml_dtypes
    n = x.shape[0]
    shard, nwin, nchunk = cfg["shard"], cfg["nwin"], cfg["nchunk"]
    chunk_wins, chunk_rows, ngrp1 = cfg["chunk_wins"], cfg["chunk_rows"], cfg["ngrp1"]

    src = np.asarray(edge_index[0], dtype=np.int64)
    dst = np.asarray(edge_index[1], dtype=np.int64)
    deg = np.bincount(dst, minlength=n).astype(np.float32)
    deg_inv = np.where(deg > 0, np.float32(1.0) / np.maximum(deg, 1.0), 0.0
                       ).astype(np.float32)

    core = dst // shard
    local = dst % shard
    win = local // P
    dstl = (local % P).astype(np.int32)

    # L1 grouping by source range block
    l1_blk_rows = [min(L1_RANGE, n - q * L1_RANGE) for q in range(ngrp1)]
    g1 = np.minimum(src // L1_RANGE, ngrp1 - 1)
    l1loc = (src - g1 * L1_RANGE).astype(np.int32)

    # L2 grouping by AllGather chunk block
    csz = chunk_wins * P
    c2 = np.minimum((src % shard) // csz, nchunk - 1)
    l2loc = ((src // shard) * np.array(chunk_rows)[c2]
             + (src % shard) - c2 * csz).astype(np.int32)
    l2_blk_rows = [NCORES * r for r in chunk_rows]

    idx1, dst1, T1, S1, colT1, colS1, sumT1, sumS1 = _pack(
        l1loc, g1, win, core, dstl, ngrp1, nwin, l1_blk_rows)
    idx2, dst2, T2, S2, colT2, colS2, sumT2, sumS2 = _pack(
        l2loc, c2, win, core, dstl, nchunk, nwin, l2_blk_rows)

    # bf16 x table with per-block zero row
    xbf = x.astype(ml_dtypes.bfloat16)
    xblocks = []
    for q in range(ngrp1):
        xb = xbf[q * L1_RANGE: q * L1_RANGE + l1_blk_rows[q]]
        xblocks.append(np.concatenate(
            [xb, np.zeros((1, x.shape[1]), ml_dtypes.bfloat16)]))
    xdev = np.concatenate(xblocks, axis=0)
    l1_base = np.concatenate([[0], np.cumsum([b.shape[0] for b in xblocks])])[:-1]

    # per-core transposed shard + deg_inv rows
    xts, dinvs = [], []
    pad = nwin * P - shard
    for ci in range(NCORES):
        xs = x[ci * shard:(ci + 1) * shard]
        xts.append(np.concatenate(
            [xs, np.zeros((pad, x.shape[1]), np.float32)]).T.copy())
        dv = np.concatenate([deg_inv[ci * shard:(ci + 1) * shard],
                             np.zeros(pad, np.float32)])
        dinvs.append(np.ascontiguousarray(
            np.broadcast_to(dv, (P, nwin * P))))

    meta = dict(T1=T1, T2=T2, S1=S1, S2=S2, colT1=colT1, colS1=colS1,
                colT2=colT2, colS2=colS2, sumT1=sumT1, sumS1=sumS1,
                sumT2=sumT2, sumS2=sumS2, l1_base=l1_base,
                l1_blk_rows=l1_blk_rows, l2_blk_rows=l2_blk_rows)
    data = dict(xdev=xdev, idx1=idx1, dst1=dst1, idx2=idx2, dst2=dst2,
                xts=xts, dinvs=dinvs)
    return meta, data


# ---------------------------------------------------------------- builder

def _build(cfg, meta, ablate=(), reps=1):
    import concourse.bacc as bacc
    import concourse.mybir as mybir
    import concourse.tile as tile

    f32 = mybir.dt.float32
    bf16 = mybir.dt.bfloat16
    i16 = mybir.dt.int16
    shard, nwin, nchunk = cfg["shard"], cfg["nwin"], cfg["nchunk"]
    chunk_wins, chunk_rows, ngrp1 = cfg["chunk_wins"], cfg["chunk_rows"], cfg["ngrp1"]
    ngc = cfg["ngc"]
    T1, T2, S1, S2 = meta["T1"], meta["T2"], meta["S1"], meta["S2"]
    colT1, colS1 = meta["colT1"], meta["colS1"]
    colT2, colS2 = meta["colT2"], meta["colS2"]
    l1_base = meta["l1_base"]
    l1_blk_rows, l2_blk_rows = meta["l1_blk_rows"], meta["l2_blk_rows"]
    xdev_rows = int(l1_base[-1] + l1_blk_rows[-1] + 1)
    npad = nwin * P

    def chunk_sizes(T, S):
        cT, cS = [], []
        for c in range(ngc):
            wlo, whi = c * GW, min((c + 1) * GW, nwin)
            cT.append(int(T[wlo:whi].sum()))
            cS.append(int(S[wlo:whi].sum()))
        return cT, cS
    cT1, cS1 = chunk_sizes(T1, S1)
    cT2, cS2 = chunk_sizes(T2, S2)

    # P2_full block offsets (each block followed by one zero row)
    p2_off = np.concatenate([[0], np.cumsum([r + 1 for r in l2_blk_rows])])
    p2_rows = int(p2_off[-1])

    # AllGather fire points: after the gather-chunk containing the AG
    # chunk's last window
    ag_after = {}
    for cag in range(nchunk):
        wlast = min((cag + 1) * chunk_wins, nwin) - 1
        ag_after.setdefault(wlast // GW, []).append(cag)

    nc = bacc.Bacc(num_swdge_queues=4)
    dp = nc.declare_dram_parameter
    xdev = dp("xdev", [xdev_rows, IN_CH], bf16, isOutput=False)
    xt = dp("xt", [P, npad], f32, isOutput=False)
    dinv = dp("dinv", [P, npad], f32, isOutput=False)
    idx1 = dp("idx1", [P, meta["sumT1"] // 16], i16, isOutput=False)
    dst1 = dp("dst1", [P, meta["sumS1"]], f32, isOutput=False)
    idx2 = dp("idx2", [P, meta["sumT2"] // 16], i16, isOutput=False)
    dst2 = dp("dst2", [P, meta["sumS2"]], f32, isOutput=False)
    w1l = dp("w1l", [IN_CH, HIDDEN], f32, isOutput=False)
    w1r = dp("w1r", [IN_CH, HIDDEN], f32, isOutput=False)
    w2l = dp("w2l", [HIDDEN, OUT_CH], f32, isOutput=False)
    w2r = dp("w2r", [HIDDEN, OUT_CH], f32, isOutput=False)
    b1c = dp("b1c", [P, 1], f32, isOutput=False)
    b2b = dp("b2b", [P, 1], f32, isOutput=False)
    iota = dp("iota", [P, P], f32, isOutput=False)
    y = dp("y", [OUT_CH, npad], f32, isOutput=True)

    p2all = nc.dram_tensor("p2all", [npad, P], bf16)
    r2d = nc.dram_tensor("r2d", [OUT_CH, npad], f32)
    p2_full = nc.dram_tensor("p2_full", [p2_rows, P], bf16,
                             addr_space="Shared")

    with tile.TileContext(nc) as tc:
        with (
            tc.tile_pool(name="const", bufs=1) as cb,
            tc.tile_pool(name="sb", bufs=2) as sb,
            tc.tile_pool(name="sm", bufs=3) as sm,
            tc.tile_pool(name="ps", bufs=2, space="PSUM") as ps,
            tc.tile_pool(name="psb", bufs=1, space="PSUM") as psb,
        ):
            # ---- constants
            def cload(param, shape, tag, dt=f32):
                t = cb.tile(shape, dt, tag=tag)
                nc.sync.dma_start(out=t[:], in_=param[:])
                return t
            iota_t = cload(iota, [P, P], "c_iota")
            w1l_t = cload(w1l, [IN_CH, HIDDEN], "c_w1l")
            w1r_t = cload(w1r, [IN_CH, HIDDEN], "c_w1r")
            w2l_t = cload(w2l, [HIDDEN, OUT_CH], "c_w2l")
            w2r_t = cload(w2r, [HIDDEN, OUT_CH], "c_w2r")
            b1_t = cload(b1c, [P, 1], "c_b1")
            b2_t = cload(b2b, [P, 1], "c_b2")
            zrow_t = cb.tile([P, P], bf16)
            nc.vector.memset(zrow_t[:], 0.0)
            mconst = cb.tile([P, P], bf16)
            nc.vector.memset(mconst[:], 0.0)

            # zero rows of p2_full (written once, before collectives run)
            for c in range(nchunk):
                zr = int(p2_off[c] + l2_blk_rows[c])
                nc.sync.dma_start(out=p2_full[zr:zr + 1, :], in_=zrow_t[:1, :])

            is_eq = mybir.AluOpType.is_equal
            copyf = mybir.ActivationFunctionType.Copy

            def build_m(mt, dt_, ns):
                """One DVE op: mt[p, g*128+n] = (iota[p,n] == dt_[p,g])."""
                in0 = (iota_t[:].rearrange("p (o n) -> p o n", o=1)
                       .to_broadcast((P, ns, P)))
                in1 = (dt_[:].rearrange("p (g o) -> p g o", o=1)
                       .to_broadcast((P, ns, P)))
                nc.vector.tensor_tensor(
                    out=mt[:].rearrange("p (g n) -> p g n", n=P),
                    in0=in0, in1=in1, op=is_eq)

            for _rep in range(reps):
                # ---------------- phase 1 ----------------
                for c in range(ngc):
                    wlo, whi = c * GW, min((c + 1) * GW, nwin)
                    nwc = whi - wlo
                    tbase = int(colT1[wlo, 0])
                    sbase_c = int(colS1[wlo, 0])
                    it = sb.tile([P, cT1[c] // 16], i16, tag="it1")
                    nc.sync.dma_start(
                        out=it[:],
                        in_=idx1[:, tbase // 16:(tbase + cT1[c]) // 16])
                    gat = sb.tile([P, cS1[c] * IN_CH], bf16, tag="g1")
                    if "nogather" in ablate:
                        nc.vector.memset(gat[:], 0.0)
                    for q in range(ngrp1):
                        t_q = sum(int(T1[w, q]) for w in range(wlo, whi))
                        if t_q == 0 or "nogather" in ablate:
                            continue
                        sseg = int(colS1[wlo, q]) - sbase_c
                        ibase = (int(colT1[wlo, q]) - tbase) // 16
                        blo = int(l1_base[q])
                        nrows = l1_blk_rows[q] + 1
                        nc.gpsimd.dma_gather(
                            out_ap=gat[:, sseg * IN_CH:
                                       (sseg + t_q // P) * IN_CH]
                            .rearrange("p (c e) -> p c e", e=IN_CH),
                            in_ap=xdev[blo:blo + nrows, :],
                            idxs_ap=it[:, ibase:ibase + t_q // 16],
                            num_idxs=t_q,
                            num_idxs_reg=t_q,
                            elem_size=IN_CH,
                            single_packet=False,
                            queue_num=0 if "q0" in ablate else q,
                        )
                    if "gatheronly" in ablate:
                        continue

                    dt_ = sb.tile([P, cS1[c]], f32, tag="dt1")
                    nc.sync.dma_start(
                        out=dt_[:], in_=dst1[:, sbase_c:sbase_c + cS1[c]])
                    xtw = sb.tile([P, nwc * P], f32, tag="xtw")
                    nc.sync.dma_start(out=xtw[:], in_=xt[:, wlo * P:whi * P])
                    dvc = sb.tile([P, nwc * P], f32, tag="dvc1")
                    nc.sync.dma_start(out=dvc[:],
                                      in_=dinv[:, wlo * P:whi * P])
                    mt = sb.tile([P, cS1[c] * P], bf16, tag="m1")
                    if "constm" not in ablate:
                        build_m(mt, dt_, cS1[c])

                    p2sb = sb.tile([P, nwc * P], bf16, tag="p2sb")
                    nc.vector.memset(p2sb[:], 0.0)
                    r2sb = sb.tile([OUT_CH, nwc * P], f32, tag="r2sb")
                    for wi in range(nwc):
                        w = wlo + wi
                        psum1 = ps.tile([P, IN_CH], f32, tag="ps1",
                                        space="PSUM")
                        slices = []
                        for q in range(ngrp1):
                            s0 = int(colS1[w, q]) - sbase_c
                            slices += list(range(s0, s0 + int(S1[w, q])))
                        for j, g in enumerate(slices):
                            m_ap = (mconst[:] if "constm" in ablate
                                    else mt[:, g * P:(g + 1) * P])
                            nc.tensor.matmul(
                                out=psum1[:],
                                lhsT=gat[:, g * IN_CH:(g + 1) * IN_CH],
                                rhs=m_ap,
                                start=(j == 0), stop=(j == len(slices) - 1))
                        # T1T = (agg * deginv)^T [f, n]
                        t1t = sm.tile([P, P], f32, tag="t1t")
                        nc.vector.tensor_tensor(
                            out=t1t[:], in0=psum1[:],
                            in1=dvc[:, wi * P:(wi + 1) * P],
                            op=mybir.AluOpType.mult)
                        # hT = relu(W1l^T T1T + W1r^T XTw + b1)  [h,n]
                        psum2 = psb.tile([P, P], f32, tag="ps2", space="PSUM")
                        nc.tensor.matmul(out=psum2[:], lhsT=w1l_t[:],
                                         rhs=t1t[:], start=True, stop=False)
                        nc.tensor.matmul(out=psum2[:], lhsT=w1r_t[:],
                                         rhs=xtw[:, wi * P:(wi + 1) * P],
                                         start=False, stop=True)
                        ht = sm.tile([P, P], f32, tag="ht")
                        nc.vector.tensor_scalar(
                            out=ht[:], in0=psum2[:], scalar1=b1_t[:, :1],
                            scalar2=0.0,
                            op0=mybir.AluOpType.add, op1=mybir.AluOpType.max)
                        # P2 rows = h @ W2_l  [n,64] -> bf16, 128-col padded
                        psum3 = psb.tile([P, OUT_CH], f32, tag="ps3",
                                         space="PSUM")
                        nc.tensor.matmul(out=psum3[:], lhsT=ht[:],
                                         rhs=w2l_t[:], start=True, stop=True)
                        nc.scalar.activation(
                            out=p2sb[:, wi * P:wi * P + OUT_CH],
                            in_=psum3[:], func=copyf)
                        # R2T = (h @ W2_r)^T + b2  [64,n]
                        psum4 = psb.tile([OUT_CH, P], f32, tag="ps4",
                                         space="PSUM")
                        nc.tensor.matmul(out=psum4[:], lhsT=w2r_t[:],
                                         rhs=ht[:], start=True, stop=True)
                        nc.vector.tensor_scalar(
                            out=r2sb[:, wi * P:(wi + 1) * P], in0=psum4[:],
                            scalar1=b2_t[:OUT_CH, :1], scalar2=None,
                            op0=mybir.AluOpType.add)

                    nc.sync.dma_start(
                        out=p2all[wlo * P:whi * P, :]
                        .rearrange("(w p) o -> p w o", p=P),
                        in_=p2sb[:].rearrange("p (w o) -> p w o", o=P))
                    nc.sync.dma_start(out=r2d[:, wlo * P:whi * P],
                                      in_=r2sb[:])

                    for cag in ag_after.get(c, []):
                        off = int(p2_off[cag])
                        rows = l2_blk_rows[cag]
                        r0 = cag * chunk_wins * P
                        if "noag" in ablate:
                            continue
                        nc.gpsimd.collective_compute(
                            "AllGather",
                            mybir.AluOpType.bypass,
                            replica_groups=[list(range(NCORES))],
                            ins=[p2all[r0:r0 + chunk_rows[cag], :]],
                            outs=[p2_full[off:off + rows, :]],
                        )

                # ---------------- phase 2 ----------------
                if "gatheronly" in ablate:
                    yz = sb.tile([OUT_CH, npad], f32, tag="yz")
                    nc.vector.memset(yz[:], 0.0)
                    nc.sync.dma_start(out=y[:], in_=yz[:])
                    continue
                for c in range(ngc):
                    wlo, whi = c * GW, min((c + 1) * GW, nwin)
                    nwc = whi - wlo
                    tbase = int(colT2[wlo, 0])
                    sbase_c = int(colS2[wlo, 0])
                    it = sb.tile([P, cT2[c] // 16], i16, tag="it2")
                    nc.sync.dma_start(
                        out=it[:],
                        in_=idx2[:, tbase // 16:(tbase + cT2[c]) // 16])
                    r2c = sb.tile([OUT_CH, nwc * P], f32, tag="r2c")
                    nc.sync.dma_start(out=r2c[:], in_=r2d[:, wlo * P:whi * P])
                    dvc = sb.tile([OUT_CH, nwc * P], f32, tag="dvc2")
                    nc.sync.dma_start(out=dvc[:],
                                      in_=dinv[:OUT_CH, wlo * P:whi * P])

                    gat = sb.tile([P, cS2[c] * P], bf16, tag="g2")
                    if "nogather" in ablate and "nophase2" not in ablate:
                        nc.vector.memset(gat[:], 0.0)
                    for q in range(nchunk):
                        t_q = sum(int(T2[w, q]) for w in range(wlo, whi))
                        if (t_q == 0 or "nogather" in ablate
                                or "nophase2" in ablate):
                            continue
                        sseg = int(colS2[wlo, q]) - sbase_c
                        ibase = (int(colT2[wlo, q]) - tbase) // 16
                        off = int(p2_off[q])
                        nrows = l2_blk_rows[q] + 1
                        nc.gpsimd.dma_gather(
                            out_ap=gat[:, sseg * P:(sseg + t_q // P) * P]
                            .rearrange("p (c e) -> p c e", e=P),
                            in_ap=p2_full[off:off + nrows, :],
                            idxs_ap=it[:, ibase:ibase + t_q // 16],
                            num_idxs=t_q,
                            num_idxs_reg=t_q,
                            elem_size=P,
                            single_packet=False,
                            queue_num=0 if "q0" in ablate else q,
                        )

                    if "nophase2" not in ablate:
                        dt_ = sb.tile([P, cS2[c]], f32, tag="dt2")
                        nc.sync.dma_start(
                            out=dt_[:], in_=dst2[:, sbase_c:sbase_c + cS2[c]])
                        mt = sb.tile([P, cS2[c] * P], bf16, tag="m2")
                        if "constm" not in ablate:
                            build_m(mt, dt_, cS2[c])

                    ysb = sb.tile([OUT_CH, nwc * P], f32, tag="ysb")
                    for wi in range(nwc):
                        w = wlo + wi
                        slices = []
                        for q in range(nchunk):
                            s0 = int(colS2[w, q]) - sbase_c
                            slices += list(range(s0, s0 + int(S2[w, q])))
                        if "nophase2" in ablate or not slices:
                            nc.vector.tensor_copy(
                                out=ysb[:, wi * P:(wi + 1) * P],
                                in_=r2c[:, wi * P:(wi + 1) * P])
                            continue
                        psum5 = ps.tile([P, P], f32, tag="ps5", space="PSUM")
                        for j, g in enumerate(slices):
                            m_ap = (mconst[:] if "constm" in ablate
                                    else mt[:, g * P:(g + 1) * P])
                            nc.tensor.matmul(
                                out=psum5[:],
                                lhsT=gat[:, g * P:(g + 1) * P],
                                rhs=m_ap,
                                start=(j == 0), stop=(j == len(slices) - 1))
                        # y = agg2 * deginv + R2
                        tmp = sm.tile([OUT_CH, P], f32, tag="ytmp")
                        nc.vector.tensor_tensor(
                            out=tmp[:], in0=psum5[:OUT_CH, :],
                            in1=dvc[:, wi * P:(wi + 1) * P],
                            op=mybir.AluOpType.mult)
                        nc.vector.tensor_add(
                            out=ysb[:, wi * P:(wi + 1) * P], in0=tmp[:],
                            in1=r2c[:, wi * P:(wi + 1) * P])
                    nc.sync.dma_start(out=y[:, wlo * P:whi * P], in_=ysb[:])

    nc.compile()
    return nc


# ---------------------------------------------------------------- entry

_CACHE = {}


def kernel(x, edge_index, W1_l, W1_r, b1, W2_l, W2_r, b2):
    x = np.asarray(x, dtype=np.float32)
    edge_index = np.asarray(edge_index)
    cfg = _derive_cfg(x.shape[0])
    meta, data = _preprocess(x, edge_index, cfg)

    key = (x.shape, edge_index.shape)
    if key in _CACHE and _CACHE[key][1] == _meta_sig(meta):
        nc = _CACHE[key][0]
    else:
        nc = _build(cfg, meta)
        _CACHE[key] = (nc, _meta_sig(meta))

    in_maps = _make_inmaps(
        dict(W1_l=W1_l, W1_r=W1_r, b1=b1, W2_l=W2_l, W2_r=W2_r, b2=b2),
        meta, data)

    from concourse.bass_utils import run_bass_kernel_spmd
    r = run_bass_kernel_spmd(nc, in_maps, core_ids=list(range(NCORES)))
    shard = cfg["shard"]
    out = np.concatenate(
        [r.results[c]["y"].T[:shard] for c in range(NCORES)], axis=0)
    return np.ascontiguousarray(out, dtype=np.float32)


def _meta_sig(meta):
    return (int(meta["sumT1"]), int(meta["sumS1"]),
            int(meta["sumT2"]), int(meta["sumS2"]))


def _make_inmaps(inputs, meta, data):
    iota_v = np.tile(np.arange(P, dtype=np.float32), (P, 1))
    common = dict(
        xdev=data["xdev"],
        w1l=np.asarray(inputs["W1_l"], np.float32),
        w1r=np.asarray(inputs["W1_r"], np.float32),
        w2l=np.asarray(inputs["W2_l"], np.float32),
        w2r=np.asarray(inputs["W2_r"], np.float32),
        b1c=np.asarray(inputs["b1"], np.float32).reshape(P, 1),
        b2b=np.concatenate([np.asarray(inputs["b2"], np.float32),
                            np.zeros(P - OUT_CH, np.float32)]).reshape(P, 1),
        iota=iota_v,
    )
    in_maps = []
    for ci in range(NCORES):
        m = dict(common)
        m["xt"] = data["xts"][ci]
        m["dinv"] = data["dinvs"][ci]
        m["idx1"] = data["idx1"][ci]
        m["dst1"] = data["dst1"][ci]
        m["idx2"] = data["idx2"][ci]
        m["dst2"] = data["dst2"][ci]
        in_maps.append(m)
    return in_maps


# revision 17
# speedup vs baseline: 1.1826x; 1.1826x over previous
"""Two-layer GraphSAGE (mean aggregation) on 8 Trainium2 NeuronCores.

Strategy (per the sharding hint): nodes + edges sharded by destination
across 8 cores; weights replicated; P2 = h @ W2_l exchanged via chunked
AllGather so layer-2 aggregation can gather any source's row.

Implementation notes (v4, evidence-driven):
  * Aggregation is per-128-edge one-hot matmuls accumulated in PSUM; edges
    are packed into GW-window gather chunks, counts padded to multiples of
    128 per (window, source-block) so every matmul slice targets exactly
    one destination window.
  * gpsimd.dma_gather descriptor generation is the bottleneck; it runs on
    one Q7 core-pair per SWDGE queue, so the 4 per-chunk source-block
    gathers are spread over queues 0-3 (4x parallel generation).
  * Layer-1 rows are gathered in bf16 (256B descriptor floor); layer-2 P2
    rows are bf16 padded to 128 cols (256B).
  * All one-hot matrices of a chunk are built with ONE DVE is_equal over
    broadcast (stride-0) access patterns -- per-slice DVE ops and their
    cross-engine handshakes were the second bottleneck.
  * deg_inv is folded in after aggregation (per window, from a per-core
    [1, npad] table) instead of being baked into the one-hots.
  * R2 (root term of layer 2) round-trips through DRAM to save SBUF.

Self-contained: hardcodes the problem shapes.
"""

import numpy as np

# ---------------------------------------------------------------- config

IN_CH, HIDDEN, OUT_CH = 128, 128, 64
N_NODES, N_EDGES = 100000, 1600000
NCORES = 8
P = 128                      # partitions / window size
L1_RANGE = 25000             # L1 gather block size (int16 limit)
GW = 1                       # windows per gather chunk


def _derive_cfg(n_nodes):
    shard = n_nodes // NCORES
    nwin = (shard + P - 1) // P
    nchunk = 4 if nwin >= 4 else 1          # AllGather chunks
    chunk_wins = (nwin + nchunk - 1) // nchunk
    chunk_rows = []
    for c in range(nchunk):
        lo = c * chunk_wins * P
        hi = min((c + 1) * chunk_wins * P, shard)
        chunk_rows.append(max(hi - lo, 0))
    ngrp1 = (n_nodes + L1_RANGE - 1) // L1_RANGE
    ngc = (nwin + GW - 1) // GW             # gather chunks
    return dict(shard=shard, nwin=nwin, nchunk=nchunk, chunk_wins=chunk_wins,
                chunk_rows=chunk_rows, ngrp1=ngrp1, ngc=ngc)


# ---------------------------------------------------------------- host prep

def _pack(src_loc, grp, win, core, dstl, ngrp, nwin, zero_rows):
    """Pack per-core gather indices + per-slice dst arrays.

    Nesting order: gather-chunk (GW windows) -> grp -> window. Counts per
    (win, grp) are padded to multiples of 128 (max over cores) so slices
    are window-pure; padding edges point at the block's zero row and carry
    dst 999 (matches no one-hot column).
    """
    ngc = (nwin + GW - 1) // GW
    key = (core * nwin + win) * ngrp + grp
    cnt = np.bincount(key, minlength=NCORES * nwin * ngrp
                      ).reshape(NCORES, nwin, ngrp)
    T128 = (cnt.max(axis=0) + P - 1) // P * P          # [nwin, ngrp]
    S = T128 // P
    sumT = int(T128.sum())
    sumS = int(S.sum())

    colT = np.zeros((nwin, ngrp), np.int64)
    colS = np.zeros((nwin, ngrp), np.int64)
    off_t = off_s = 0
    for c in range(ngc):
        for q in range(ngrp):
            for w in range(c * GW, min((c + 1) * GW, nwin)):
                colT[w, q] = off_t
                colS[w, q] = off_s
                off_t += int(T128[w, q])
                off_s += int(S[w, q])
    assert off_t == sumT and off_s == sumS

    idx_all = np.zeros((NCORES, 16, sumT // 16), np.int16)
    dst_all = np.full((NCORES, P, sumS), 999.0, np.float32)

    chunkidx = win // GW
    order = np.lexsort((win, grp, chunkidx, core))
    so, go, wo, co = (a[order] for a in (src_loc, grp, win, core))
    dl_o = dstl[order]
    e0 = 0
    for ci in range(NCORES):
        for c in range(ngc):
            for q in range(ngrp):
                for w in range(c * GW, min((c + 1) * GW, nwin)):
                    k = int(cnt[ci, w, q])
                    t = int(T128[w, q])
                    if t == 0:
                        assert k == 0
                        continue
                    buf = np.full(t, zero_rows[q], np.int32)
                    buf[:k] = so[e0:e0 + k]
                    db = np.full(t, 999.0, np.float32)
                    db[:k] = dl_o[e0:e0 + k]
                    e0 += k
                    base = int(colT[w, q]) // 16
                    idx_all[ci, :, base:base + t // 16] = (
                        buf.reshape(t // 16, 16).T)
                    ns = t // P
                    sbase = int(colS[w, q])
                    dst_all[ci, :, sbase:sbase + ns] = db.reshape(ns, P).T
    assert e0 == len(order)
    idx_rep = np.tile(idx_all, (1, 8, 1))
    return idx_rep, dst_all, T128, S, colT, colS, sumT, sumS


def _preprocess(x, edge_index, cfg):
    import ml_dtypes
    n = x.shape[0]
    shard, nwin, nchunk = cfg["shard"], cfg["nwin"], cfg["nchunk"]
    chunk_wins, chunk_rows, ngrp1 = cfg["chunk_wins"], cfg["chunk_rows"], cfg["ngrp1"]

    src = np.asarray(edge_index[0], dtype=np.int64)
    dst = np.asarray(edge_index[1], dtype=np.int64)
    deg = np.bincount(dst, minlength=n).astype(np.float32)
    deg_inv = np.where(deg > 0, np.float32(1.0) / np.maximum(deg, 1.0), 0.0
                       ).astype(np.float32)

    core = dst // shard
    local = dst % shard
    win = local // P
    dstl = (local % P).astype(np.int32)

    # L1 grouping by source range block
    l1_blk_rows = [min(L1_RANGE, n - q * L1_RANGE) for q in range(ngrp1)]
    g1 = np.minimum(src // L1_RANGE, ngrp1 - 1)
    l1loc = (src - g1 * L1_RANGE).astype(np.int32)

    # L2 grouping by AllGather chunk block
    csz = chunk_wins * P
    c2 = np.minimum((src % shard) // csz, nchunk - 1)
    l2loc = ((src // shard) * np.array(chunk_rows)[c2]
             + (src % shard) - c2 * csz).astype(np.int32)
    l2_blk_rows = [NCORES * r for r in chunk_rows]

    idx1, dst1, T1, S1, colT1, colS1, sumT1, sumS1 = _pack(
        l1loc, g1, win, core, dstl, ngrp1, nwin, l1_blk_rows)
    idx2, dst2, T2, S2, colT2, colS2, sumT2, sumS2 = _pack(
        l2loc, c2, win, core, dstl, nchunk, nwin, l2_blk_rows)

    # bf16 x table with per-block zero row
    xbf = x.astype(ml_dtypes.bfloat16)
    xblocks = []
    for q in range(ngrp1):
        xb = xbf[q * L1_RANGE: q * L1_RANGE + l1_blk_rows[q]]
        xblocks.append(np.concatenate(
            [xb, np.zeros((1, x.shape[1]), ml_dtypes.bfloat16)]))
    xdev = np.concatenate(xblocks, axis=0)
    l1_base = np.concatenate([[0], np.cumsum([b.shape[0] for b in xblocks])])[:-1]

    # per-core transposed shard + deg_inv rows
    xts, dinvs = [], []
    pad = nwin * P - shard
    for ci in range(NCORES):
        xs = x[ci * shard:(ci + 1) * shard]
        xts.append(np.concatenate(
            [xs, np.zeros((pad, x.shape[1]), np.float32)]).T.copy())
        dv = np.concatenate([deg_inv[ci * shard:(ci + 1) * shard],
                             np.zeros(pad, np.float32)])
        dinvs.append(np.ascontiguousarray(
            np.broadcast_to(dv, (P, nwin * P))))

    meta = dict(T1=T1, T2=T2, S1=S1, S2=S2, colT1=colT1, colS1=colS1,
                colT2=colT2, colS2=colS2, sumT1=sumT1, sumS1=sumS1,
                sumT2=sumT2, sumS2=sumS2, l1_base=l1_base,
                l1_blk_rows=l1_blk_rows, l2_blk_rows=l2_blk_rows)
    data = dict(xdev=xdev, idx1=idx1, dst1=dst1, idx2=idx2, dst2=dst2,
                xts=xts, dinvs=dinvs)
    return meta, data


# ---------------------------------------------------------------- builder

def _build(cfg, meta, ablate=(), reps=1):
    import concourse.bacc as bacc
    import concourse.mybir as mybir
    import concourse.tile as tile

    f32 = mybir.dt.float32
    bf16 = mybir.dt.bfloat16
    i16 = mybir.dt.int16
    shard, nwin, nchunk = cfg["shard"], cfg["nwin"], cfg["nchunk"]
    chunk_wins, chunk_rows, ngrp1 = cfg["chunk_wins"], cfg["chunk_rows"], cfg["ngrp1"]
    ngc = cfg["ngc"]
    T1, T2, S1, S2 = meta["T1"], meta["T2"], meta["S1"], meta["S2"]
    colT1, colS1 = meta["colT1"], meta["colS1"]
    colT2, colS2 = meta["colT2"], meta["colS2"]
    l1_base = meta["l1_base"]
    l1_blk_rows, l2_blk_rows = meta["l1_blk_rows"], meta["l2_blk_rows"]
    xdev_rows = int(l1_base[-1] + l1_blk_rows[-1] + 1)
    npad = nwin * P

    def chunk_sizes(T, S):
        cT, cS = [], []
        for c in range(ngc):
            wlo, whi = c * GW, min((c + 1) * GW, nwin)
            cT.append(int(T[wlo:whi].sum()))
            cS.append(int(S[wlo:whi].sum()))
        return cT, cS
    cT1, cS1 = chunk_sizes(T1, S1)
    cT2, cS2 = chunk_sizes(T2, S2)

    # P2_full block offsets (each block followed by one zero row)
    p2_off = np.concatenate([[0], np.cumsum([r + 1 for r in l2_blk_rows])])
    p2_rows = int(p2_off[-1])

    # AllGather fire points: after the gather-chunk containing the AG
    # chunk's last window
    ag_after = {}
    for cag in range(nchunk):
        wlast = min((cag + 1) * chunk_wins, nwin) - 1
        ag_after.setdefault(wlast // GW, []).append(cag)

    nc = bacc.Bacc(num_swdge_queues=4)
    dp = nc.declare_dram_parameter
    xdev = dp("xdev", [xdev_rows, IN_CH], bf16, isOutput=False)
    xt = dp("xt", [P, npad], f32, isOutput=False)
    dinv = dp("dinv", [P, npad], f32, isOutput=False)
    idx1 = dp("idx1", [P, meta["sumT1"] // 16], i16, isOutput=False)
    dst1 = dp("dst1", [P, meta["sumS1"]], f32, isOutput=False)
    idx2 = dp("idx2", [P, meta["sumT2"] // 16], i16, isOutput=False)
    dst2 = dp("dst2", [P, meta["sumS2"]], f32, isOutput=False)
    w1l = dp("w1l", [IN_CH, HIDDEN], f32, isOutput=False)
    w1r = dp("w1r", [IN_CH, HIDDEN], f32, isOutput=False)
    w2l = dp("w2l", [HIDDEN, OUT_CH], f32, isOutput=False)
    w2r = dp("w2r", [HIDDEN, OUT_CH], f32, isOutput=False)
    b1c = dp("b1c", [P, 1], f32, isOutput=False)
    b2b = dp("b2b", [P, 1], f32, isOutput=False)
    iota = dp("iota", [P, P], f32, isOutput=False)
    y = dp("y", [OUT_CH, npad], f32, isOutput=True)

    p2all = nc.dram_tensor("p2all", [npad, P], bf16)
    r2d = nc.dram_tensor("r2d", [OUT_CH, npad], f32)
    p2_full = nc.dram_tensor("p2_full", [p2_rows, P], bf16,
                             addr_space="Shared")

    with tile.TileContext(nc) as tc:
        with (
            tc.tile_pool(name="const", bufs=1) as cb,
            tc.tile_pool(name="sb", bufs=3) as sb,
            tc.tile_pool(name="sm", bufs=3) as sm,
            tc.tile_pool(name="ps", bufs=2, space="PSUM") as ps,
            tc.tile_pool(name="psb", bufs=1, space="PSUM") as psb,
        ):
            # ---- constants
            def cload(param, shape, tag, dt=f32):
                t = cb.tile(shape, dt, tag=tag)
                nc.sync.dma_start(out=t[:], in_=param[:])
                return t
            iota_t = cload(iota, [P, P], "c_iota")
            w1l_t = cload(w1l, [IN_CH, HIDDEN], "c_w1l")
            w1r_t = cload(w1r, [IN_CH, HIDDEN], "c_w1r")
            w2l_t = cload(w2l, [HIDDEN, OUT_CH], "c_w2l")
            w2r_t = cload(w2r, [HIDDEN, OUT_CH], "c_w2r")
            b1_t = cload(b1c, [P, 1], "c_b1")
            b2_t = cload(b2b, [P, 1], "c_b2")
            zrow_t = cb.tile([P, P], bf16)
            nc.vector.memset(zrow_t[:], 0.0)
            mconst = cb.tile([P, P], bf16)
            nc.vector.memset(mconst[:], 0.0)

            # zero rows of p2_full (written once, before collectives run)
            for c in range(nchunk):
                zr = int(p2_off[c] + l2_blk_rows[c])
                nc.sync.dma_start(out=p2_full[zr:zr + 1, :], in_=zrow_t[:1, :])

            is_eq = mybir.AluOpType.is_equal
            copyf = mybir.ActivationFunctionType.Copy

            def build_m(mt, dt_, ns):
                """One DVE op: mt[p, g*128+n] = (iota[p,n] == dt_[p,g])."""
                in0 = (iota_t[:].rearrange("p (o n) -> p o n", o=1)
                       .to_broadcast((P, ns, P)))
                in1 = (dt_[:].rearrange("p (g o) -> p g o", o=1)
                       .to_broadcast((P, ns, P)))
                nc.vector.tensor_tensor(
                    out=mt[:].rearrange("p (g n) -> p g n", n=P),
                    in0=in0, in1=in1, op=is_eq)

            for _rep in range(reps):
                # ---------------- phase 1 ----------------
                for c in range(ngc):
                    wlo, whi = c * GW, min((c + 1) * GW, nwin)
                    nwc = whi - wlo
                    tbase = int(colT1[wlo, 0])
                    sbase_c = int(colS1[wlo, 0])
                    it = sb.tile([P, cT1[c] // 16], i16, tag="it1")
                    nc.sync.dma_start(
                        out=it[:],
                        in_=idx1[:, tbase // 16:(tbase + cT1[c]) // 16])
                    gat = sb.tile([P, cS1[c] * IN_CH], bf16, tag="g1")
                    if "nogather" in ablate:
                        nc.vector.memset(gat[:], 0.0)
                    for q in range(ngrp1):
                        t_q = sum(int(T1[w, q]) for w in range(wlo, whi))
                        if t_q == 0 or "nogather" in ablate:
                            continue
                        sseg = int(colS1[wlo, q]) - sbase_c
                        ibase = (int(colT1[wlo, q]) - tbase) // 16
                        blo = int(l1_base[q])
                        nrows = l1_blk_rows[q] + 1
                        nc.gpsimd.dma_gather(
                            out_ap=gat[:, sseg * IN_CH:
                                       (sseg + t_q // P) * IN_CH]
                            .rearrange("p (c e) -> p c e", e=IN_CH),
                            in_ap=xdev[blo:blo + nrows, :],
                            idxs_ap=it[:, ibase:ibase + t_q // 16],
                            num_idxs=t_q,
                            num_idxs_reg=t_q,
                            elem_size=IN_CH,
                            single_packet=False,
                            queue_num=0 if "q0" in ablate else q,
                        )
                    if "gatheronly" in ablate:
                        continue

                    dt_ = sb.tile([P, cS1[c]], f32, tag="dt1")
                    nc.sync.dma_start(
                        out=dt_[:], in_=dst1[:, sbase_c:sbase_c + cS1[c]])
                    xtw = sb.tile([P, nwc * P], f32, tag="xtw")
                    nc.sync.dma_start(out=xtw[:], in_=xt[:, wlo * P:whi * P])
                    dvc = sb.tile([P, nwc * P], f32, tag="dvc1")
                    nc.sync.dma_start(out=dvc[:],
                                      in_=dinv[:, wlo * P:whi * P])
                    mt = sb.tile([P, cS1[c] * P], bf16, tag="m1")
                    if "constm" not in ablate:
                        build_m(mt, dt_, cS1[c])

                    p2sb = sb.tile([P, nwc * P], bf16, tag="p2sb")
                    nc.vector.memset(p2sb[:], 0.0)
                    r2sb = sb.tile([OUT_CH, nwc * P], f32, tag="r2sb")
                    for wi in range(nwc):
                        w = wlo + wi
                        psum1 = ps.tile([P, IN_CH], f32, tag="ps1",
                                        space="PSUM")
                        slices = []
                        for q in range(ngrp1):
                            s0 = int(colS1[w, q]) - sbase_c
                            slices += list(range(s0, s0 + int(S1[w, q])))
                        for j, g in enumerate(slices):
                            m_ap = (mconst[:] if "constm" in ablate
                                    else mt[:, g * P:(g + 1) * P])
                            nc.tensor.matmul(
                                out=psum1[:],
                                lhsT=gat[:, g * IN_CH:(g + 1) * IN_CH],
                                rhs=m_ap,
                                start=(j == 0), stop=(j == len(slices) - 1))
                        # T1T = (agg * deginv)^T [f, n]
                        t1t = sm.tile([P, P], f32, tag="t1t")
                        nc.vector.tensor_tensor(
                            out=t1t[:], in0=psum1[:],
                            in1=dvc[:, wi * P:(wi + 1) * P],
                            op=mybir.AluOpType.mult)
                        # hT = relu(W1l^T T1T + W1r^T XTw + b1)  [h,n]
                        psum2 = psb.tile([P, P], f32, tag="ps2", space="PSUM")
                        nc.tensor.matmul(out=psum2[:], lhsT=w1l_t[:],
                                         rhs=t1t[:], start=True, stop=False)
                        nc.tensor.matmul(out=psum2[:], lhsT=w1r_t[:],
                                         rhs=xtw[:, wi * P:(wi + 1) * P],
                                         start=False, stop=True)
                        ht = sm.tile([P, P], f32, tag="ht")
                        nc.vector.tensor_scalar(
                            out=ht[:], in0=psum2[:], scalar1=b1_t[:, :1],
                            scalar2=0.0,
                            op0=mybir.AluOpType.add, op1=mybir.AluOpType.max)
                        # P2 rows = h @ W2_l  [n,64] -> bf16, 128-col padded
                        psum3 = psb.tile([P, OUT_CH], f32, tag="ps3",
                                         space="PSUM")
                        nc.tensor.matmul(out=psum3[:], lhsT=ht[:],
                                         rhs=w2l_t[:], start=True, stop=True)
                        nc.scalar.activation(
                            out=p2sb[:, wi * P:wi * P + OUT_CH],
                            in_=psum3[:], func=copyf)
                        # R2T = (h @ W2_r)^T + b2  [64,n]
                        psum4 = psb.tile([OUT_CH, P], f32, tag="ps4",
                                         space="PSUM")
                        nc.tensor.matmul(out=psum4[:], lhsT=w2r_t[:],
                                         rhs=ht[:], start=True, stop=True)
                        nc.vector.tensor_scalar(
                            out=r2sb[:, wi * P:(wi + 1) * P], in0=psum4[:],
                            scalar1=b2_t[:OUT_CH, :1], scalar2=None,
                            op0=mybir.AluOpType.add)

                    nc.sync.dma_start(
                        out=p2all[wlo * P:whi * P, :]
                        .rearrange("(w p) o -> p w o", p=P),
                        in_=p2sb[:].rearrange("p (w o) -> p w o", o=P))
                    nc.sync.dma_start(out=r2d[:, wlo * P:whi * P],
                                      in_=r2sb[:])

                    for cag in ag_after.get(c, []):
                        off = int(p2_off[cag])
                        rows = l2_blk_rows[cag]
                        r0 = cag * chunk_wins * P
                        if "noag" in ablate:
                            continue
                        nc.gpsimd.collective_compute(
                            "AllGather",
                            mybir.AluOpType.bypass,
                            replica_groups=[list(range(NCORES))],
                            ins=[p2all[r0:r0 + chunk_rows[cag], :]],
                            outs=[p2_full[off:off + rows, :]],
                        )

                # ---------------- phase 2 ----------------
                if "gatheronly" in ablate:
                    yz = sb.tile([OUT_CH, npad], f32, tag="yz")
                    nc.vector.memset(yz[:], 0.0)
                    nc.sync.dma_start(out=y[:], in_=yz[:])
                    continue
                for c in range(ngc):
                    wlo, whi = c * GW, min((c + 1) * GW, nwin)
                    nwc = whi - wlo
                    tbase = int(colT2[wlo, 0])
                    sbase_c = int(colS2[wlo, 0])
                    it = sb.tile([P, cT2[c] // 16], i16, tag="it2")
                    nc.sync.dma_start(
                        out=it[:],
                        in_=idx2[:, tbase // 16:(tbase + cT2[c]) // 16])
                    r2c = sb.tile([OUT_CH, nwc * P], f32, tag="r2c")
                    nc.sync.dma_start(out=r2c[:], in_=r2d[:, wlo * P:whi * P])
                    dvc = sb.tile([OUT_CH, nwc * P], f32, tag="dvc2")
                    nc.sync.dma_start(out=dvc[:],
                                      in_=dinv[:OUT_CH, wlo * P:whi * P])

                    gat = sb.tile([P, cS2[c] * P], bf16, tag="g2")
                    if "nogather" in ablate and "nophase2" not in ablate:
                        nc.vector.memset(gat[:], 0.0)
                    for q in range(nchunk):
                        t_q = sum(int(T2[w, q]) for w in range(wlo, whi))
                        if (t_q == 0 or "nogather" in ablate
                                or "nophase2" in ablate):
                            continue
                        sseg = int(colS2[wlo, q]) - sbase_c
                        ibase = (int(colT2[wlo, q]) - tbase) // 16
                        off = int(p2_off[q])
                        nrows = l2_blk_rows[q] + 1
                        nc.gpsimd.dma_gather(
                            out_ap=gat[:, sseg * P:(sseg + t_q // P) * P]
                            .rearrange("p (c e) -> p c e", e=P),
                            in_ap=p2_full[off:off + nrows, :],
                            idxs_ap=it[:, ibase:ibase + t_q // 16],
                            num_idxs=t_q,
                            num_idxs_reg=t_q,
                            elem_size=P,
                            single_packet=False,
                            queue_num=0 if "q0" in ablate else q,
                        )

                    if "nophase2" not in ablate:
                        dt_ = sb.tile([P, cS2[c]], f32, tag="dt2")
                        nc.sync.dma_start(
                            out=dt_[:], in_=dst2[:, sbase_c:sbase_c + cS2[c]])
                        mt = sb.tile([P, cS2[c] * P], bf16, tag="m2")
                        if "constm" not in ablate:
                            build_m(mt, dt_, cS2[c])

                    ysb = sb.tile([OUT_CH, nwc * P], f32, tag="ysb")
                    for wi in range(nwc):
                        w = wlo + wi
                        slices = []
                        for q in range(nchunk):
                            s0 = int(colS2[w, q]) - sbase_c
                            slices += list(range(s0, s0 + int(S2[w, q])))
                        if "nophase2" in ablate or not slices:
                            nc.vector.tensor_copy(
                                out=ysb[:, wi * P:(wi + 1) * P],
                                in_=r2c[:, wi * P:(wi + 1) * P])
                            continue
                        psum5 = ps.tile([P, P], f32, tag="ps5", space="PSUM")
                        for j, g in enumerate(slices):
                            m_ap = (mconst[:] if "constm" in ablate
                                    else mt[:, g * P:(g + 1) * P])
                            nc.tensor.matmul(
                                out=psum5[:],
                                lhsT=gat[:, g * P:(g + 1) * P],
                                rhs=m_ap,
                                start=(j == 0), stop=(j == len(slices) - 1))
                        # y = agg2 * deginv + R2
                        tmp = sm.tile([OUT_CH, P], f32, tag="ytmp")
                        nc.vector.tensor_tensor(
                            out=tmp[:], in0=psum5[:OUT_CH, :],
                            in1=dvc[:, wi * P:(wi + 1) * P],
                            op=mybir.AluOpType.mult)
                        nc.vector.tensor_add(
                            out=ysb[:, wi * P:(wi + 1) * P], in0=tmp[:],
                            in1=r2c[:, wi * P:(wi + 1) * P])
                    nc.sync.dma_start(out=y[:, wlo * P:whi * P], in_=ysb[:])

    nc.compile()
    return nc


# ---------------------------------------------------------------- entry

_CACHE = {}


def kernel(x, edge_index, W1_l, W1_r, b1, W2_l, W2_r, b2):
    x = np.asarray(x, dtype=np.float32)
    edge_index = np.asarray(edge_index)
    cfg = _derive_cfg(x.shape[0])
    meta, data = _preprocess(x, edge_index, cfg)

    key = (x.shape, edge_index.shape)
    if key in _CACHE and _CACHE[key][1] == _meta_sig(meta):
        nc = _CACHE[key][0]
    else:
        nc = _build(cfg, meta)
        _CACHE[key] = (nc, _meta_sig(meta))

    in_maps = _make_inmaps(
        dict(W1_l=W1_l, W1_r=W1_r, b1=b1, W2_l=W2_l, W2_r=W2_r, b2=b2),
        meta, data)

    from concourse.bass_utils import run_bass_kernel_spmd
    r = run_bass_kernel_spmd(nc, in_maps, core_ids=list(range(NCORES)))
    shard = cfg["shard"]
    out = np.concatenate(
        [r.results[c]["y"].T[:shard] for c in range(NCORES)], axis=0)
    return np.ascontiguousarray(out, dtype=np.float32)


def _meta_sig(meta):
    return (int(meta["sumT1"]), int(meta["sumS1"]),
            int(meta["sumT2"]), int(meta["sumS2"]))


def _make_inmaps(inputs, meta, data):
    iota_v = np.tile(np.arange(P, dtype=np.float32), (P, 1))
    common = dict(
        xdev=data["xdev"],
        w1l=np.asarray(inputs["W1_l"], np.float32),
        w1r=np.asarray(inputs["W1_r"], np.float32),
        w2l=np.asarray(inputs["W2_l"], np.float32),
        w2r=np.asarray(inputs["W2_r"], np.float32),
        b1c=np.asarray(inputs["b1"], np.float32).reshape(P, 1),
        b2b=np.concatenate([np.asarray(inputs["b2"], np.float32),
                            np.zeros(P - OUT_CH, np.float32)]).reshape(P, 1),
        iota=iota_v,
    )
    in_maps = []
    for ci in range(NCORES):
        m = dict(common)
        m["xt"] = data["xts"][ci]
        m["dinv"] = data["dinvs"][ci]
        m["idx1"] = data["idx1"][ci]
        m["dst1"] = data["dst1"][ci]
        m["idx2"] = data["idx2"][ci]
        m["dst2"] = data["dst2"][ci]
        in_maps.append(m)
    return in_maps
